# revision 3
# baseline (speedup 1.0000x reference)
"""Chamfer distance + F1 kernel for Trainium2 (8 NeuronCores).

Strategy (B=4 batches, N=M=8192 points, 3D):
  - core c handles batch b = c//2, row-half h = c%2 of xyz1 (4096 rows).
  - PE computes the scaled squared-distance block 4096*d[p,f] in ONE fp16
    matmul pass using an augmented K=13 contraction built on the host:
    each fp32 operand is split into fp16 hi+lo parts so the result is
    accurate to ~1e-6 while streaming at full bf16/fp16 PE rate.
  - A custom fused DVE op (MINCOLROW) consumes each distance tile ONCE:
    out = min(in0, in1) updates the running column-min accumulator M,
    while the accumulator port folds the row-min of in0 (accum_out).
    It is registered with BOTH a REGULAR (1x) and a hand-authored
    2X_1PORT uop program; perf_max=1 on the instruction lets the engine
    engage 2 elem/lane/cycle when operands are contiguous fp16.
  - Per row tile, 3 of 4 PSUM granules are converted PSUM->fp16 by the
    Scalar engine (Relu) and consumed by the fused op at 2x; the 4th
    granule is consumed straight from PSUM at 1x. This balances
    Scalar ~5.6us/tile against Vector ~5.5us/tile instead of making the
    Vector engine touch every element twice (the old MIN+MINMIN 283us).
  - PE transposes M, DVE reduces -> per-column partial mins (dist2 half).
  - Host combines the two halves per batch and computes cd_p/cd_t/f1 on
    the 8192-element min vectors (0.01% of the FLOPs).
"""

import sys

if "/opt/trn_rl_repo" not in sys.path:
    sys.path.insert(0, "/opt/trn_rl_repo")

from contextlib import ExitStack

import numpy as np

import concourse.tile as tile
import concourse.dve_ops as dve_ops
from concourse import bacc, mybir
from concourse.bass_utils import run_bass_kernel_spmd
from concourse.dve_spec import C0, AluOp, Spec, Src0, Src1, lower, minn
from concourse.dve_uop import (
    AluInp,
    DelayInp,
    DveOpSpec,
    InpSel,
    Trigger,
    UopConfig,
    OutPath,
    OutSel,
)

F16 = mybir.dt.float16
F32 = mybir.dt.float32
MIN = mybir.AluOpType.min
AXX = mybir.AxisListType.X

SCALE = 24.0  # coordinate prescale; distances come out scaled by SCALE**2
# (max pairwise sq-dist for these inputs is ~92; 92*24^2 = 53k < fp16 max)
DSCALE = SCALE * SCALE
F1_THRESHOLD = 1e-4
FP16_MAX = 65504.0

N_CORES = 8
K_AUG = 13  # 9 coord-product rows + 2 sq1 rows + 2 sq2 rows


def _split16(v):
    """Split fp32 array into fp16 hi + lo so hi+lo ~= v to ~2^-22 rel."""
    hi = v.astype(np.float16)
    lo = (v - hi.astype(np.float32)).astype(np.float16)
    return hi, lo


def _prep_core(xyz1_half, xyz2_full):
    """Build the augmented fp16 operands for one core.

    Returns lhsT [13, n_rows] (stationary, xyz1 side) and
    rhs [13, n_cols] (moving, xyz2 side) such that
    sum_k lhsT[k,p] * rhs[k,f] ~= DSCALE * ||xyz1[p] - xyz2[f]||^2.
    """
    n_rows = xyz1_half.shape[0]
    n_cols = xyz2_full.shape[0]
    v1 = (-2.0 * SCALE) * xyz1_half.astype(np.float32)  # [n_rows, 3]
    w2 = SCALE * xyz2_full.astype(np.float32)  # [n_cols, 3]
    h1, l1 = _split16(v1)
    h2, l2 = _split16(w2)

    s1 = (SCALE * xyz1_half.astype(np.float32)) ** 2
    s1q = s1.sum(axis=1) * 0.25  # DSCALE*sq1 / 4
    s2q = ((w2.astype(np.float64) ** 2).sum(axis=1) * 0.25).astype(np.float32)
    s1h, s1l = _split16(s1q)
    s2h, s2l = _split16(s2q)

    lhsT = np.empty((K_AUG, n_rows), np.float16)
    rhs = np.empty((K_AUG, n_cols), np.float16)
    for c in range(3):
        lhsT[3 * c + 0] = h1[:, c]
        lhsT[3 * c + 1] = h1[:, c]
        lhsT[3 * c + 2] = l1[:, c]
        rhs[3 * c + 0] = h2[:, c]
        rhs[3 * c + 1] = l2[:, c]
        rhs[3 * c + 2] = h2[:, c]
    lhsT[9] = s1h
    lhsT[10] = s1l
    rhs[9] = np.float16(4.0)
    rhs[10] = np.float16(4.0)
    lhsT[11] = np.float16(4.0)
    lhsT[12] = np.float16(4.0)
    rhs[11] = s2h
    rhs[12] = s2l
    return lhsT, rhs


# lane layout for the hand-authored 2X_1PORT program
_L_S0, _L_S1, _L_C0, _L_S0H, _L_S1H, _L_CAP = 0, 1, 2, 3, 4, 5


def _mincolrow_2x_uops():
    """2X_1PORT program: per cycle consume fp16 pairs (lo, hi) of in0/in1.

      blk0: lo_min  = MIN(s0_lo, s1_lo)
      blk1: hi_min  = MIN(s0_hi, s1_hi);      lane5 <- lo_min
      blk2: pairmin = MIN(s0_lo, s0_hi);      lane1 <- hi_min
      blk3: acc     = MIN(acc, pairmin)  (out_a tail)
      out:  WR0_LO <- lane5 (lo_min), WR0_HI <- lane1 (hi_min)

    State machine mirrors lower()'s [seed, steady]: seed writes the
    accumulator's C0 init once, steady streams until SRC_TENSOR_DONE.
    """
    P = AluInp.PREV_ALU_OUT
    D = AluInp.PREV_DELAY_0

    def inputs(u):
        u.enable_input(InpSel.SRC_0, _L_S0 + 1)
        u.enable_input(InpSel.SRC_1, _L_S1 + 1)
        u.enable_input(InpSel.CONST_0, _L_C0 + 1)
        u.enable_input(InpSel.SRC_0_HI, _L_S0H + 1)
        u.enable_input(InpSel.SRC_1_HI, _L_S1H + 1)
        return u

    seed = inputs(UopConfig())
    seed.trigger = (Trigger.COUNT, Trigger.NONE, Trigger.NONE)
    seed.repeat_count = 1
    seed.next_uop = (1, 0, 0)
    seed.accum_enabled = 1
    for b in range(3):
        seed.datapath_config[b].pass_through_alu()
        seed.datapath_config[b].pass_through_delay(_L_C0)
    seed.datapath_config[3].enable_alu(
        AluOp.BYPASS, AluInp(int(D) + _L_C0), AluInp(int(D) + _L_C0)
    )
    seed.datapath_config[3].alu_out_a_enable = 1
    for b in range(4, 8):
        seed.datapath_config[b].pass_through_alu()
        seed.datapath_config[b].alu_out_a_enable = 1

    st = inputs(UopConfig())
    st.trigger = (Trigger.SRC_TENSOR_DONE, Trigger.NONE, Trigger.NONE)
    st.require_inp0 = 1
    st.require_inp1 = 1
    st.accum_enabled = 1
    dp = st.datapath_config
    dp[0].enable_alu(AluOp.MIN, AluInp(int(D) + _L_S0), AluInp(int(D) + _L_S1))
    dp[0].pass_through_delay(_L_S0, _L_C0, _L_S0H, _L_S1H)
    dp[1].enable_alu(AluOp.MIN, AluInp(int(D) + _L_S0H), AluInp(int(D) + _L_S1H))
    dp[1].pass_through_delay(_L_S0, _L_C0, _L_S0H)
    dp[1].enable_delay_from_src(DelayInp.PREV_ALU_OUT, _L_CAP)
    dp[2].enable_alu(AluOp.MIN, AluInp(int(D) + _L_S0), AluInp(int(D) + _L_S0H))
    dp[2].pass_through_delay(_L_C0, _L_CAP)
    dp[2].enable_delay_from_src(DelayInp.PREV_ALU_OUT, _L_S1)
    dp[3].enable_alu(AluOp.MIN, AluInp.CURR_ALU_OUT, P)
    dp[3].alu_out_a_enable = 1
    dp[3].pass_through_delay(_L_S1, _L_C0, _L_CAP)
    for b in range(4, 8):
        dp[b].pass_through_alu()
        dp[b].alu_out_a_enable = 1
        dp[b].pass_through_delay(_L_S1, _L_C0, _L_CAP)
    st.enable_output(OutSel.DELAY_5, OutPath.WR0_LO)
    st.enable_output(OutSel.DELAY_1, OutPath.WR0_HI)
    return [seed, st]


def _register_mincolrow():
    """Fused custom DVE op: out = min(in0, in1); accum_out = min-fold(in0).

    One pass per distance tile updates the running column-min (out) AND
    produces the tile's row-min (accum_out). REGULAR program is lower()'s
    min+accum output with the accumulator operand re-muxed from the body
    result to the Src0 delay lane (fold in0, not min(in0,in1)). The
    2X_1PORT program is hand-authored (see _mincolrow_2x_uops).
    """
    name = "MINCOLROW_ANT"
    if name in dve_ops._SUB_OPCODE_FOR_NAME:
        return next(op for op in dve_ops.OPS if op.name == name)

    def _ref(in0, in1, c0, c1, c2):
        a0 = np.asarray(in0, np.float32)
        out = np.minimum(a0, np.asarray(in1, np.float32))
        acc = a0.reshape(a0.shape[0], -1).min(axis=-1, keepdims=True)
        acc = np.minimum(acc, c0)
        return out, acc

    spec = Spec(body=minn(Src0, Src1), accum=AluOp.MIN, accum_init=C0,
                reference=_ref)
    uops1 = lower(spec, ver="v3")
    steady = uops1[-1]
    src0_lane = next(
        ln for ln in range(6)
        if steady.inp_enable[ln + 1] and steady.inp[ln + 1] == InpSel.SRC_0
    )
    # accum block (1) folds Src0's lane instead of the body result
    steady.datapath_config[1].alu_src1 = AluInp(int(AluInp.PREV_DELAY_0) + src0_lane)

    row = max(dve_ops._SUB_OPCODE_FOR_NAME.values()) + 1
    spec_obj = DveOpSpec(
        name=name, opcode=row, uops=uops1, uops_2x=_mincolrow_2x_uops(),
        perf_max=1, rd1_en=True,
    )
    spec_obj.validate("v3")

    op = dve_ops.DveOp(name, spec, subdim=False, uops_sha={})
    dve_ops.OPS.append(op)
    dve_ops.CUSTOM_DVE_SPECS[name] = spec
    dve_ops._SUB_OPCODE_FOR_NAME[name] = row
    dve_ops._COMPILE_CACHE[(name, "v3")] = spec_obj
    return op


# flip to 0 to force every MINCOLROW call onto the REGULAR (1x) program
PERF_MAX = 1


def build_program(n_rows=4096, n_cols=8192):
    """Build + compile the per-core Bass program (same program on all cores)."""
    ROWT = n_rows // 128  # row tiles (32)
    CG = 2048  # granule = 4 PSUM banks
    NG = n_cols // CG  # granules per row tile (4)
    NA = NG - 1  # granules converted by ScalarE (3); last goes DVE-direct
    WACT = NA * CG  # fp16-converted width per tile (6144)
    NB = n_cols // 128  # 128-col blocks for the transpose tail
    PER = min(16, NB)  # transpose blocks per PSUM tile

    MCR = _register_mincolrow()
    nc = bacc.Bacc("TRN2", target_bir_lowering=False, debug=False,
                   num_devices=N_CORES)
    lhsT_d = nc.dram_tensor("lhsT", [K_AUG, n_rows], F16, kind="ExternalInput").ap()
    rhs_d = nc.dram_tensor("rhs", [K_AUG, n_cols], F16, kind="ExternalInput").ap()
    id_d = nc.dram_tensor("ident", [128, 128], F16, kind="ExternalInput").ap()
    out1_d = nc.dram_tensor("out1", [128, ROWT], F16, kind="ExternalOutput").ap()
    out2_d = nc.dram_tensor("out2", [128, NB], F32, kind="ExternalOutput").ap()

    with tile.TileContext(nc) as tc, ExitStack() as ctx:
        const = ctx.enter_context(tc.tile_pool(name="const", bufs=1))
        d16p = ctx.enter_context(tc.tile_pool(name="d16", bufs=3))
        mp = ctx.enter_context(tc.tile_pool(name="m875", bufs=1))
        outp = ctx.enter_context(tc.tile_pool(name="outs", bufs=1))
        psp = ctx.enter_context(tc.tile_pool(name="ps", bufs=2, space="PSUM"))

        w_sb = const.tile([K_AUG, n_rows], F16)
        nc.sync.dma_start(w_sb[:], lhsT_d)
        r_sb = const.tile([K_AUG, n_cols], F16)
        # chunked so the first matmuls only wait on their own slice; the
        # leading chunks are small to light up the PE pipe early
        s = 0
        for w in [512, 512, 1024] + [CG] * (n_cols // CG):
            if s >= n_cols:
                break
            w = min(w, n_cols - s)
            nc.sync.dma_start(r_sb[:, s:s + w], rhs_d[:, s:s + w])
            s += w
        id_sb = const.tile([128, 128], F16)
        nc.sync.dma_start(id_sb[:], id_d)

        M = mp.tile([128, n_cols], F16)
        Ra = outp.tile([128, ROWT], F16)
        Rb = outp.tile([128, ROWT], F16)
        Rf = outp.tile([128, ROWT], F16)
        C = outp.tile([128, NB], F32)
        nc.vector.memset(M[:], FP16_MAX)

        for t in range(ROWT):
            d16 = d16p.tile([128, WACT], F16, tag="d16")
            # ScalarE granules: PSUM -> fp16 (Relu clamps rounding negatives)
            for g in range(NA):
                ps = psp.tile([128, CG], F32, tag="ps")
                for j in range(CG // 512):
                    c0 = g * CG + 512 * j
                    nc.tensor.matmul(
                        ps[:, 512 * j:512 * (j + 1)],
                        w_sb[:, 128 * t:128 * (t + 1)],
                        r_sb[:, c0:c0 + 512],
                        start=True, stop=True,
                    )
                nc.scalar.activation(
                    d16[:, g * CG:(g + 1) * CG], ps[:],
                    mybir.ActivationFunctionType.Relu,
                )
            # direct granule: fused op reads PSUM fp32 at 1x
            gs = NA * CG
            ps = psp.tile([128, CG], F32, tag="ps")
            for j in range(CG // 512):
                nc.tensor.matmul(
                    ps[:, 512 * j:512 * (j + 1)],
                    w_sb[:, 128 * t:128 * (t + 1)],
                    r_sb[:, gs + 512 * j:gs + 512 * (j + 1)],
                    start=True, stop=True,
                )
            i1 = nc.vector._custom_dve(
                MCR, out=M[:, gs:n_cols], in0=ps[:], in1=M[:, gs:n_cols],
                s0=FP16_MAX, accum_out=Rb[:, t:t + 1],
            )
            i1.perf_max = PERF_MAX
            # fused op over the converted region at 2x
            i2 = nc.vector._custom_dve(
                MCR, out=M[:, 0:WACT], in0=d16[:], in1=M[:, 0:WACT],
                s0=FP16_MAX, accum_out=Ra[:, t:t + 1],
            )
            i2.perf_max = PERF_MAX

        nc.vector.tensor_tensor(Rf[:], Ra[:], Rb[:], op=MIN)

        # column-min of M across its 128 partitions: PE-transpose 128-col
        # blocks into PSUM (manual start/stop: 8 fp16 blocks share a bank),
        # then reduce along the transposed free dim.
        for q in range(NB // PER):
            psT = psp.tile([128, PER * 128], F16, tag="ps")
            for j in range(PER):
                blk = q * PER + j
                nc.tensor.matmul(
                    psT[:, 128 * j:128 * (j + 1)],
                    M[:, 128 * blk:128 * (blk + 1)],
                    id_sb[:],
                    is_transpose=True,
                    start=(j % 8 == 0), stop=(j % 8 == 7),
                )
            nc.vector.tensor_reduce(
                C[:, PER * q:PER * (q + 1)],
                psT[:].rearrange("p (b c) -> p b c", c=128),
                axis=AXX, op=MIN,
            )

        nc.sync.dma_start(out1_d, Rf[:])
        nc.sync.dma_start(out2_d, C[:])

    nc.compile()
    # the Tile scheduler re-emits instructions, dropping perf_max set at
    # trace time — patch the scheduled module instead. Only fp16-in0 calls
    # qualify for 2X_1PORT; fp32/PSUM calls stay REGULAR.
    if PERF_MAX:
        for b in nc.m.functions[0].blocks:
            for i in b.instructions:
                if (getattr(i, "op_name", None) == MCR.name
                        and i.ins[0].dtype == F16):
                    i.perf_max = PERF_MAX
    return nc


_CACHE = {}


def _get_program(n_rows, n_cols):
    key = (n_rows, n_cols)
    if key not in _CACHE:
        _CACHE[key] = build_program(n_rows, n_cols)
    return _CACHE[key]


def run_device(xyz1, xyz2, trace=False):
    """Run the 8-core SPMD program; returns (dist1 [B,N], dist2 [B,M], results)."""
    xyz1 = np.asarray(xyz1)
    xyz2 = np.asarray(xyz2)
    B, N, _ = xyz1.shape
    M = xyz2.shape[1]
    halves = N_CORES // B  # row-halves per batch (2)
    n_rows = N // halves
    nc = _get_program(n_rows, M)

    ident = np.eye(128, dtype=np.float16)
    in_maps = []
    for c in range(N_CORES):
        b, h = divmod(c, halves)
        lhsT, rhs = _prep_core(
            xyz1[b, h * n_rows:(h + 1) * n_rows], xyz2[b])
        in_maps.append({"lhsT": lhsT, "rhs": rhs, "ident": ident})

    res = run_bass_kernel_spmd(nc, in_maps, list(range(N_CORES)), trace=trace)

    dist1 = np.empty((B, N), np.float64)
    dist2p = np.empty((B, halves, M), np.float64)
    for c in range(N_CORES):
        b, h = divmod(c, halves)
        o1 = res.results[c]["out1"].astype(np.float64)  # [128, ROWT]
        o2 = res.results[c]["out2"].astype(np.float64)  # [128, NB]
        dist1[b, h * n_rows:(h + 1) * n_rows] = o1.T.reshape(-1)
        dist2p[b, h] = o2.T.reshape(-1)
    # direct-PSUM granules skip the Relu clamp; tiny negatives are possible
    dist1 = np.maximum(dist1, 0.0) / DSCALE
    dist2 = np.maximum(dist2p.min(axis=1), 0.0) / DSCALE
    return dist1, dist2, res


def _finalize(dist1, dist2):
    cd_p = (np.sqrt(dist1).mean(axis=1) + np.sqrt(dist2).mean(axis=1)) / 2.0
    cd_t = dist1.mean(axis=1) + dist2.mean(axis=1)
    p1 = (dist1 < F1_THRESHOLD).mean(axis=1)
    p2 = (dist2 < F1_THRESHOLD).mean(axis=1)
    denom = p1 + p2
    f1 = np.where(denom > 0, 2.0 * p1 * p2 / np.where(denom > 0, denom, 1.0), 0.0)
    return (cd_p.astype(np.float32), cd_t.astype(np.float32),
            f1.astype(np.float32))


def kernel(xyz1, xyz2):
    dist1, dist2, _ = run_device(xyz1, xyz2, trace=False)
    return _finalize(dist1, dist2)


# revision 4
# speedup vs baseline: 1.0106x; 1.0106x over previous
"""Chamfer distance + F1 kernel for Trainium2 (8 NeuronCores).

Strategy (B=4 batches, N=M=8192 points, 3D):
  - core c handles batch b = c//2, row-half h = c%2 of xyz1 (4096 rows).
  - PE computes the scaled squared-distance block 4096*d[p,f] in ONE fp16
    matmul pass using an augmented K=13 contraction built on the host:
    each fp32 operand is split into fp16 hi+lo parts so the result is
    accurate to ~1e-6 while streaming at full bf16/fp16 PE rate.
  - A custom fused DVE op (MINCOLROW) consumes each distance tile ONCE:
    out = min(in0, in1) updates the running column-min accumulator M,
    while the accumulator port folds the row-min of in0 (accum_out).
    It is registered with BOTH a REGULAR (1x) and a hand-authored
    2X_1PORT uop program; perf_max=1 on the instruction lets the engine
    engage 2 elem/lane/cycle when operands are contiguous fp16.
  - Per row tile, 3 of 4 PSUM granules are converted PSUM->fp16 by the
    Scalar engine (Relu) and consumed by the fused op at 2x; the 4th
    granule is consumed straight from PSUM at 1x. This balances
    Scalar ~5.6us/tile against Vector ~5.5us/tile instead of making the
    Vector engine touch every element twice (the old MIN+MINMIN 283us).
  - PE transposes M, DVE reduces -> per-column partial mins (dist2 half).
  - Host combines the two halves per batch and computes cd_p/cd_t/f1 on
    the 8192-element min vectors (0.01% of the FLOPs).
"""

import sys

if "/opt/trn_rl_repo" not in sys.path:
    sys.path.insert(0, "/opt/trn_rl_repo")

from contextlib import ExitStack

import numpy as np

import concourse.tile as tile
import concourse.dve_ops as dve_ops
from concourse import bacc, mybir
from concourse.bass_utils import run_bass_kernel_spmd
from concourse.dve_spec import C0, AluOp, Spec, Src0, Src1, lower, minn
from concourse.dve_uop import (
    AluInp,
    DelayInp,
    DveOpSpec,
    InpSel,
    Trigger,
    UopConfig,
    OutPath,
    OutSel,
)

F16 = mybir.dt.float16
F32 = mybir.dt.float32
MIN = mybir.AluOpType.min
AXX = mybir.AxisListType.X

SCALE = 24.0  # coordinate prescale; distances come out scaled by SCALE**2
# (max pairwise sq-dist for these inputs is ~92; 92*24^2 = 53k < fp16 max)
DSCALE = SCALE * SCALE
F1_THRESHOLD = 1e-4
FP16_MAX = 65504.0

N_CORES = 8
K_AUG = 13  # 9 coord-product rows + 2 sq1 rows + 2 sq2 rows


def _split16(v):
    """Split fp32 array into fp16 hi + lo so hi+lo ~= v to ~2^-22 rel."""
    hi = v.astype(np.float16)
    lo = (v - hi.astype(np.float32)).astype(np.float16)
    return hi, lo


def _prep_core(xyz1_half, xyz2_full):
    """Build the augmented fp16 operands for one core.

    Returns lhsT [13, n_rows] (stationary, xyz1 side) and
    rhs [13, n_cols] (moving, xyz2 side) such that
    sum_k lhsT[k,p] * rhs[k,f] ~= DSCALE * ||xyz1[p] - xyz2[f]||^2.
    """
    n_rows = xyz1_half.shape[0]
    n_cols = xyz2_full.shape[0]
    v1 = (-2.0 * SCALE) * xyz1_half.astype(np.float32)  # [n_rows, 3]
    w2 = SCALE * xyz2_full.astype(np.float32)  # [n_cols, 3]
    h1, l1 = _split16(v1)
    h2, l2 = _split16(w2)

    s1 = (SCALE * xyz1_half.astype(np.float32)) ** 2
    s1q = s1.sum(axis=1) * 0.25  # DSCALE*sq1 / 4
    s2q = ((w2.astype(np.float64) ** 2).sum(axis=1) * 0.25).astype(np.float32)
    s1h, s1l = _split16(s1q)
    s2h, s2l = _split16(s2q)

    lhsT = np.empty((K_AUG, n_rows), np.float16)
    rhs = np.empty((K_AUG, n_cols), np.float16)
    for c in range(3):
        lhsT[3 * c + 0] = h1[:, c]
        lhsT[3 * c + 1] = h1[:, c]
        lhsT[3 * c + 2] = l1[:, c]
        rhs[3 * c + 0] = h2[:, c]
        rhs[3 * c + 1] = l2[:, c]
        rhs[3 * c + 2] = h2[:, c]
    lhsT[9] = s1h
    lhsT[10] = s1l
    rhs[9] = np.float16(4.0)
    rhs[10] = np.float16(4.0)
    lhsT[11] = np.float16(4.0)
    lhsT[12] = np.float16(4.0)
    rhs[11] = s2h
    rhs[12] = s2l
    return lhsT, rhs


# lane layout for the hand-authored 2X_1PORT program
_L_S0, _L_S1, _L_C0, _L_S0H, _L_S1H, _L_CAP = 0, 1, 2, 3, 4, 5


def _mincolrow_2x_uops():
    """2X_1PORT program: per cycle consume fp16 pairs (lo, hi) of in0/in1.

      blk0: lo_min  = MIN(s0_lo, s1_lo)
      blk1: hi_min  = MIN(s0_hi, s1_hi);      lane5 <- lo_min
      blk2: pairmin = MIN(s0_lo, s0_hi);      lane1 <- hi_min
      blk3: acc     = MIN(acc, pairmin)  (out_a tail)
      out:  WR0_LO <- lane5 (lo_min), WR0_HI <- lane1 (hi_min)

    State machine mirrors lower()'s [seed, steady]: seed writes the
    accumulator's C0 init once, steady streams until SRC_TENSOR_DONE.
    """
    P = AluInp.PREV_ALU_OUT
    D = AluInp.PREV_DELAY_0

    def inputs(u):
        u.enable_input(InpSel.SRC_0, _L_S0 + 1)
        u.enable_input(InpSel.SRC_1, _L_S1 + 1)
        u.enable_input(InpSel.CONST_0, _L_C0 + 1)
        u.enable_input(InpSel.SRC_0_HI, _L_S0H + 1)
        u.enable_input(InpSel.SRC_1_HI, _L_S1H + 1)
        return u

    seed = inputs(UopConfig())
    seed.trigger = (Trigger.COUNT, Trigger.NONE, Trigger.NONE)
    seed.repeat_count = 1
    seed.next_uop = (1, 0, 0)
    seed.accum_enabled = 1
    for b in range(3):
        seed.datapath_config[b].pass_through_alu()
        seed.datapath_config[b].pass_through_delay(_L_C0)
    seed.datapath_config[3].enable_alu(
        AluOp.BYPASS, AluInp(int(D) + _L_C0), AluInp(int(D) + _L_C0)
    )
    seed.datapath_config[3].alu_out_a_enable = 1
    for b in range(4, 8):
        seed.datapath_config[b].pass_through_alu()
        seed.datapath_config[b].alu_out_a_enable = 1

    st = inputs(UopConfig())
    st.trigger = (Trigger.SRC_TENSOR_DONE, Trigger.NONE, Trigger.NONE)
    st.require_inp0 = 1
    st.require_inp1 = 1
    st.accum_enabled = 1
    dp = st.datapath_config
    dp[0].enable_alu(AluOp.MIN, AluInp(int(D) + _L_S0), AluInp(int(D) + _L_S1))
    dp[0].pass_through_delay(_L_S0, _L_C0, _L_S0H, _L_S1H)
    dp[1].enable_alu(AluOp.MIN, AluInp(int(D) + _L_S0H), AluInp(int(D) + _L_S1H))
    dp[1].pass_through_delay(_L_S0, _L_C0, _L_S0H)
    dp[1].enable_delay_from_src(DelayInp.PREV_ALU_OUT, _L_CAP)
    dp[2].enable_alu(AluOp.MIN, AluInp(int(D) + _L_S0), AluInp(int(D) + _L_S0H))
    dp[2].pass_through_delay(_L_C0, _L_CAP)
    dp[2].enable_delay_from_src(DelayInp.PREV_ALU_OUT, _L_S1)
    dp[3].enable_alu(AluOp.MIN, AluInp.CURR_ALU_OUT, P)
    dp[3].alu_out_a_enable = 1
    dp[3].pass_through_delay(_L_S1, _L_C0, _L_CAP)
    for b in range(4, 8):
        dp[b].pass_through_alu()
        dp[b].alu_out_a_enable = 1
        dp[b].pass_through_delay(_L_S1, _L_C0, _L_CAP)
    st.enable_output(OutSel.DELAY_5, OutPath.WR0_LO)
    st.enable_output(OutSel.DELAY_1, OutPath.WR0_HI)
    return [seed, st]


def _register_mincolrow():
    """Fused custom DVE op: out = min(in0, in1); accum_out = min-fold(in0).

    One pass per distance tile updates the running column-min (out) AND
    produces the tile's row-min (accum_out). REGULAR program is lower()'s
    min+accum output with the accumulator operand re-muxed from the body
    result to the Src0 delay lane (fold in0, not min(in0,in1)). The
    2X_1PORT program is hand-authored (see _mincolrow_2x_uops).
    """
    name = "MINCOLROW_ANT"
    if name in dve_ops._SUB_OPCODE_FOR_NAME:
        return next(op for op in dve_ops.OPS if op.name == name)

    def _ref(in0, in1, c0, c1, c2):
        a0 = np.asarray(in0, np.float32)
        out = np.minimum(a0, np.asarray(in1, np.float32))
        acc = a0.reshape(a0.shape[0], -1).min(axis=-1, keepdims=True)
        acc = np.minimum(acc, c0)
        return out, acc

    spec = Spec(body=minn(Src0, Src1), accum=AluOp.MIN, accum_init=C0,
                reference=_ref)
    uops1 = lower(spec, ver="v3")
    steady = uops1[-1]
    src0_lane = next(
        ln for ln in range(6)
        if steady.inp_enable[ln + 1] and steady.inp[ln + 1] == InpSel.SRC_0
    )
    # accum block (1) folds Src0's lane instead of the body result
    steady.datapath_config[1].alu_src1 = AluInp(int(AluInp.PREV_DELAY_0) + src0_lane)

    row = max(dve_ops._SUB_OPCODE_FOR_NAME.values()) + 1
    spec_obj = DveOpSpec(
        name=name, opcode=row, uops=uops1, uops_2x=_mincolrow_2x_uops(),
        perf_max=1, rd1_en=True,
    )
    spec_obj.validate("v3")

    op = dve_ops.DveOp(name, spec, subdim=False, uops_sha={})
    dve_ops.OPS.append(op)
    dve_ops.CUSTOM_DVE_SPECS[name] = spec
    dve_ops._SUB_OPCODE_FOR_NAME[name] = row
    dve_ops._COMPILE_CACHE[(name, "v3")] = spec_obj
    return op


# flip to 0 to force every MINCOLROW call onto the REGULAR (1x) program
PERF_MAX = 1


def build_program(n_rows=4096, n_cols=8192):
    """Build + compile the per-core Bass program (same program on all cores)."""
    ROWT = n_rows // 128  # row tiles (32)
    CG = 2048  # granule = 4 PSUM banks
    NG = n_cols // CG  # granules per row tile (4)
    NA = NG - 1  # granules converted by ScalarE (3); last goes DVE-direct
    WACT = NA * CG  # fp16-converted width per tile (6144)
    NB = n_cols // 128  # 128-col blocks for the transpose tail
    PER = min(16, NB)  # transpose blocks per PSUM tile

    MCR = _register_mincolrow()
    nc = bacc.Bacc("TRN2", target_bir_lowering=False, debug=False,
                   num_devices=N_CORES)
    lhsT_d = nc.dram_tensor("lhsT", [K_AUG, n_rows], F16, kind="ExternalInput").ap()
    rhs_d = nc.dram_tensor("rhs", [K_AUG, n_cols], F16, kind="ExternalInput").ap()
    id_d = nc.dram_tensor("ident", [128, 128], F16, kind="ExternalInput").ap()
    out1_d = nc.dram_tensor("out1", [128, ROWT], F16, kind="ExternalOutput").ap()
    out2_d = nc.dram_tensor("out2", [128, NB], F32, kind="ExternalOutput").ap()

    with tile.TileContext(nc) as tc, ExitStack() as ctx:
        const = ctx.enter_context(tc.tile_pool(name="const", bufs=1))
        d16p = ctx.enter_context(tc.tile_pool(name="d16", bufs=3))
        mp = ctx.enter_context(tc.tile_pool(name="m875", bufs=1))
        outp = ctx.enter_context(tc.tile_pool(name="outs", bufs=1))
        psp = ctx.enter_context(tc.tile_pool(name="ps", bufs=2, space="PSUM"))

        w_sb = const.tile([K_AUG, n_rows], F16)
        nc.sync.dma_start(w_sb[:], lhsT_d)
        r_sb = const.tile([K_AUG, n_cols], F16)
        # chunked so the first matmuls only wait on their own slice; the
        # leading chunks are small to light up the PE pipe early
        s = 0
        for w in [512, 512, 1024] + [CG] * (n_cols // CG):
            if s >= n_cols:
                break
            w = min(w, n_cols - s)
            nc.sync.dma_start(r_sb[:, s:s + w], rhs_d[:, s:s + w])
            s += w
        id_sb = const.tile([128, 128], F16)
        nc.sync.dma_start(id_sb[:], id_d)

        M = mp.tile([128, n_cols], F16)
        Ra = outp.tile([128, ROWT], F16)
        Rb = outp.tile([128, ROWT], F16)
        Rf = outp.tile([128, ROWT], F16)
        C = outp.tile([128, NB], F32)
        nc.vector.memset(M[:], FP16_MAX)

        for t in range(ROWT):
            d16 = d16p.tile([128, WACT], F16, tag="d16")
            # ScalarE granules: PSUM -> fp16 (Relu clamps rounding negatives)
            for g in range(NA):
                ps = psp.tile([128, CG], F32, tag="ps")
                for j in range(CG // 512):
                    c0 = g * CG + 512 * j
                    nc.tensor.matmul(
                        ps[:, 512 * j:512 * (j + 1)],
                        w_sb[:, 128 * t:128 * (t + 1)],
                        r_sb[:, c0:c0 + 512],
                        start=True, stop=True,
                    )
                nc.scalar.activation(
                    d16[:, g * CG:(g + 1) * CG], ps[:],
                    mybir.ActivationFunctionType.Relu,
                )
            # direct granule: fused op reads PSUM fp32 at 1x
            gs = NA * CG
            ps = psp.tile([128, CG], F32, tag="ps")
            for j in range(CG // 512):
                nc.tensor.matmul(
                    ps[:, 512 * j:512 * (j + 1)],
                    w_sb[:, 128 * t:128 * (t + 1)],
                    r_sb[:, gs + 512 * j:gs + 512 * (j + 1)],
                    start=True, stop=True,
                )
            i1 = nc.vector._custom_dve(
                MCR, out=M[:, gs:n_cols], in0=ps[:], in1=M[:, gs:n_cols],
                s0=FP16_MAX, accum_out=Rb[:, t:t + 1],
            )
            i1.perf_max = PERF_MAX
            # fused op over the converted region at 2x
            i2 = nc.vector._custom_dve(
                MCR, out=M[:, 0:WACT], in0=d16[:], in1=M[:, 0:WACT],
                s0=FP16_MAX, accum_out=Ra[:, t:t + 1],
            )
            i2.perf_max = PERF_MAX

        nc.vector.tensor_tensor(Rf[:], Ra[:], Rb[:], op=MIN)

        # column-min of M across its 128 partitions: PE-transpose 128-col
        # blocks into PSUM (manual start/stop: 8 fp16 blocks share a bank),
        # then reduce along the transposed free dim.
        for q in range(NB // PER):
            psT = psp.tile([128, PER * 128], F16, tag="ps")
            for j in range(PER):
                blk = q * PER + j
                nc.tensor.matmul(
                    psT[:, 128 * j:128 * (j + 1)],
                    M[:, 128 * blk:128 * (blk + 1)],
                    id_sb[:],
                    is_transpose=True,
                    start=(j % 8 == 0), stop=(j % 8 == 7),
                )
            nc.vector.tensor_reduce(
                C[:, PER * q:PER * (q + 1)],
                psT[:].rearrange("p (b c) -> p b c", c=128),
                axis=AXX, op=MIN,
            )

        nc.sync.dma_start(out1_d, Rf[:])
        nc.sync.dma_start(out2_d, C[:])

    nc.compile()
    # the Tile scheduler re-emits instructions and the 64-byte ISA encoding
    # is baked at creation, so set perf_max (byte 36 bits [7:6]) directly in
    # the scheduled module's instruction bytes. Only fp16-in0 calls qualify
    # for 2X_1PORT; fp32/PSUM calls stay REGULAR.
    if PERF_MAX:
        for b in nc.m.functions[0].blocks:
            for i in b.instructions:
                if (getattr(i, "op_name", None) == MCR.name
                        and i.ins[0].dtype == F16):
                    raw = i.instr
                    raw[36] |= PERF_MAX << 6
                    i.instr = raw
                    i.perf_max = PERF_MAX
    return nc


_CACHE = {}


def _get_program(n_rows, n_cols):
    key = (n_rows, n_cols)
    if key not in _CACHE:
        _CACHE[key] = build_program(n_rows, n_cols)
    return _CACHE[key]


def run_device(xyz1, xyz2, trace=False):
    """Run the 8-core SPMD program; returns (dist1 [B,N], dist2 [B,M], results)."""
    xyz1 = np.asarray(xyz1)
    xyz2 = np.asarray(xyz2)
    B, N, _ = xyz1.shape
    M = xyz2.shape[1]
    halves = N_CORES // B  # row-halves per batch (2)
    n_rows = N // halves
    nc = _get_program(n_rows, M)

    ident = np.eye(128, dtype=np.float16)
    in_maps = []
    for c in range(N_CORES):
        b, h = divmod(c, halves)
        lhsT, rhs = _prep_core(
            xyz1[b, h * n_rows:(h + 1) * n_rows], xyz2[b])
        in_maps.append({"lhsT": lhsT, "rhs": rhs, "ident": ident})

    res = run_bass_kernel_spmd(nc, in_maps, list(range(N_CORES)), trace=trace)

    dist1 = np.empty((B, N), np.float64)
    dist2p = np.empty((B, halves, M), np.float64)
    for c in range(N_CORES):
        b, h = divmod(c, halves)
        o1 = res.results[c]["out1"].astype(np.float64)  # [128, ROWT]
        o2 = res.results[c]["out2"].astype(np.float64)  # [128, NB]
        dist1[b, h * n_rows:(h + 1) * n_rows] = o1.T.reshape(-1)
        dist2p[b, h] = o2.T.reshape(-1)
    # direct-PSUM granules skip the Relu clamp; tiny negatives are possible
    dist1 = np.maximum(dist1, 0.0) / DSCALE
    dist2 = np.maximum(dist2p.min(axis=1), 0.0) / DSCALE
    return dist1, dist2, res


def _finalize(dist1, dist2):
    cd_p = (np.sqrt(dist1).mean(axis=1) + np.sqrt(dist2).mean(axis=1)) / 2.0
    cd_t = dist1.mean(axis=1) + dist2.mean(axis=1)
    p1 = (dist1 < F1_THRESHOLD).mean(axis=1)
    p2 = (dist2 < F1_THRESHOLD).mean(axis=1)
    denom = p1 + p2
    f1 = np.where(denom > 0, 2.0 * p1 * p2 / np.where(denom > 0, denom, 1.0), 0.0)
    return (cd_p.astype(np.float32), cd_t.astype(np.float32),
            f1.astype(np.float32))


def kernel(xyz1, xyz2):
    dist1, dist2, _ = run_device(xyz1, xyz2, trace=False)
    return _finalize(dist1, dist2)
